# revision 7
# baseline (speedup 1.0000x reference)
"""nn_Conductor: 2-layer LSTM (H=1024, T=32, B=512) on 8 TRN2 NeuronCores.

Weight-stationary tensor parallelism over the hidden dim (8-way, like the
baseline) with the batch pipelined in two 256-wide chunks. Core k owns
hidden units [k*128,(k+1)*128) of both layers, keeps all its lhsT weight
slices (bf16, 4MB) in SBUF, and computes its gate slice for the full
batch. The per-layer hidden exchange is an 8-rank AllGather.

Measured on this pod: an 8-rank AllGather costs ~13ns/KB of output with
no fixed cost (13.4us @ 1MB out, 6.7us @ 512KB), so splitting the
exchange into two per-batch-chunk AllGathers is free in wire time but
lets chunk A's AG launch while chunk B's matmuls still stream -- and
the next layer-step consumes chunk A first. Each layer-step emits
[h-part A, x-part A, tail A, h-part B, x-part B, tail B]: the h-part
rhs was gathered a full layer-step earlier, so it covers the in-flight
AGs, the act/trigger tail hides under the other chunk's matmuls, and
the PE stream never gaps long enough to lose the HAM clock (the
original single-AG baseline spent 1.9ms of its 2.4ms throttled at half
clock with 34% of the wall exposed collective time; this version
measures ~1.55ms single-exec / ~1.41-1.51ms reps-slope).

Details per layer-step per core: 128 matmuls [K=128, M=128, N=256]
(bf16, fp32 PSUM), 8 PSUM banks = 4 gates x 2 chunks double-buffered
across steps by the pool allocator, 10 scalar activations, ~10 DVE
elementwise ops, 2x 64KB SBUF->HBM bounces on the scalar HWDGE ring
(keeping them out of the sync ring where the gather DMAs wait on their
AG - HWDGE rings are FIFO per issuing engine), 2 AllGathers, and per AG
three segment-gathers (256+128+128KB) so the low k-tiles unblock the
next x-part early and the rest arrive in consumption order. The
final timestep's layer-1 exchange is skipped (no consumer). Init loads
are pre-transposed host-side into the exact SBUF images so they are a
handful of full-bandwidth contiguous DMAs. State c stays fp32;
end-to-end rel err ~4e-3 vs the fp32 reference.
"""
import sys

sys.path.insert(0, "/opt/trn_rl_repo")

import numpy as np
from ml_dtypes import bfloat16

import concourse.bacc as bacc
import concourse.mybir as mybir
import concourse.tile as tile

H, L, T, B = 1024, 2, 32, 512
P = 128
NC = 8
KT = H // P          # 8 k-tiles
S = H // NC          # 128 hidden units per core
NCH = 2              # batch chunks
BC = B // NCH        # 256 batch per chunk
F32 = mybir.dt.float32
BF16 = mybir.dt.bfloat16
Sig = mybir.ActivationFunctionType.Sigmoid
Tanh = mybir.ActivationFunctionType.Tanh


def _prep_inputs(z, to_h_W, to_h_b, init_emb, W_ih, W_hh, b_ih, b_hh):
    z = np.asarray(z, np.float32)
    to_h_W = np.asarray(to_h_W, np.float32)
    to_h_b = np.asarray(to_h_b, np.float32)
    init_emb = np.asarray(init_emb, np.float32)
    W_ih = np.asarray(W_ih, np.float32)
    W_hh = np.asarray(W_hh, np.float32)
    b_comb = np.asarray(b_ih, np.float32) + np.asarray(b_hh, np.float32)

    zT = np.ascontiguousarray(
        z.T.reshape(KT, P, B).transpose(1, 0, 2).reshape(P, KT * B)
    ).astype(bfloat16)
    x0T = np.ascontiguousarray(
        np.broadcast_to(init_emb[0][:, None], (H, B))
        .reshape(KT, P, B).transpose(1, 0, 2).reshape(P, KT * B)
    ).astype(bfloat16)
    tohT = np.ascontiguousarray(to_h_W.T)
    toh_blocks = np.empty((2 * KT, KT, P, P), np.float32)
    for mi in range(2 * KT):
        for kt in range(KT):
            toh_blocks[mi, kt] = tohT[kt * P:(kt + 1) * P, mi * P:(mi + 1) * P]
    # pre-transposed to the SBUF image [P, cols] so init loads are a single
    # full-bandwidth contiguous DMA instead of hundreds of 256B-line ones
    toh_arr = np.ascontiguousarray(
        toh_blocks.transpose(2, 0, 1, 3).reshape(P, 2 * KT * KT * P)
    ).astype(bfloat16)
    tohb = np.ascontiguousarray(to_h_b.reshape(2 * KT, P).T).astype(np.float32)

    in_maps = []
    for k in range(NC):
        w_blocks = np.empty((4, KT, P, 4 * P), np.float32)
        for mat_i, Wfull in enumerate((W_ih[0], W_hh[0], W_ih[1], W_hh[1])):
            rows = np.concatenate(
                [Wfull[q * H + k * S: q * H + k * S + P, :] for q in range(4)],
                axis=0)
            WT = np.ascontiguousarray(rows.T)
            for kt in range(KT):
                w_blocks[mat_i, kt] = WT[kt * P:(kt + 1) * P, :]
        w_arr = np.ascontiguousarray(
            w_blocks.reshape(4 * KT, P, 4 * P).transpose(1, 0, 2)
            .reshape(P, 4 * KT * 4 * P)
        ).astype(bfloat16)
        b_arr = np.empty((P, 8), np.float32)
        for l in range(L):
            for q in range(4):
                b_arr[:, l * 4 + q] = b_comb[l, q * H + k * S: q * H + k * S + P]
        in_maps.append({"w": w_arr, "b": b_arr, "toh": toh_arr, "tohb": tohb,
                        "zT": zT, "x0T": x0T})
    return in_maps


def _build(reps=1, use_ag=True):
    nc = bacc.Bacc("TRN2", target_bir_lowering=False, debug=False)

    w_ext = nc.declare_dram_parameter("w", [P, 4 * KT * 4 * P], BF16, isOutput=False)
    b_ext = nc.declare_dram_parameter("b", [P, 8], F32, isOutput=False)
    toh_ext = nc.declare_dram_parameter("toh", [P, 2 * KT * KT * P], BF16, isOutput=False)
    tohb_ext = nc.declare_dram_parameter("tohb", [P, 2 * KT], F32, isOutput=False)
    zT_ext = nc.declare_dram_parameter("zT", [P, KT * B], BF16, isOutput=False)
    x0T_ext = nc.declare_dram_parameter("x0T", [P, KT * B], BF16, isOutput=False)
    ys_ext = nc.declare_dram_parameter("ys", [T * P, B], F32, isOutput=True)

    with tile.TileContext(nc) as tc:
        with (
            tc.tile_pool(name="const", bufs=1) as const,
            tc.tile_pool(name="state", bufs=1) as state,
            tc.tile_pool(name="hT", bufs=6) as hTp,
            tc.tile_pool(name="act", bufs=2) as actp,
            tc.tile_pool(name="psum", bufs=8, space="PSUM") as psum,
            tc.tile_pool(name="dram", bufs=4, space="DRAM") as dram,
        ):
            tohb_sb = const.tile([P, 2 * KT], F32)
            nc.sync.dma_start(tohb_sb[:], tohb_ext[:])
            zT_sb = const.tile([P, KT * B], BF16)
            nc.sync.dma_start(zT_sb[:], zT_ext[:])
            # toh in 4 chunks so the first h0-init matmuls start while the
            # rest of the 4MB still streams
            toh_sb = const.tile([P, 2 * KT * KT * P], BF16)
            CH4 = 2 * KT * KT * P // 4
            for c4 in range(4):
                nc.sync.dma_start(toh_sb[:, c4 * CH4:(c4 + 1) * CH4],
                                  toh_ext[:, c4 * CH4:(c4 + 1) * CH4])
            b_sb = const.tile([P, 8], F32)
            nc.sync.dma_start(b_sb[:], b_ext[:])
            x0T_sb = const.tile([P, KT * B], BF16)
            nc.sync.dma_start(x0T_sb[:], x0T_ext[:])
            w_sb = const.tile([P, 4 * KT * 4 * P], BF16)
            nc.sync.dma_start(w_sb[:], w_ext[:])

            # c state: [layer][chunk] of [P, BC] fp32
            c_sb = [[state.tile([P, BC], F32, tag=f"c{l}{ch}", name=f"c{l}{ch}")
                     for ch in range(NCH)] for l in range(L)]
            for l in range(L):
                for ch in range(NCH):
                    nc.any.memset(c_sb[l][ch][:], 0.0)

            # h0 init: tanh(z @ toh.T + b), written per (chunk, k-segment)
            # into the same split layout the gathered h uses.
            SEGS = ((0, 4), (4, 2), (6, 2))       # (first k-tile, n k-tiles)

            def seg_of(kt):
                for si, (k0, kl) in enumerate(SEGS):
                    if k0 <= kt < k0 + kl:
                        return si, kt - k0
                raise ValueError(kt)

            h_init = [[[hTp.tile([P, kl * BC], BF16, tag=f"hT{ch}{si}",
                                 name=f"hi{l}{ch}{si}")
                        for si, (k0, kl) in enumerate(SEGS)]
                       for ch in range(NCH)] for l in range(L)]
            for mi in range(2 * KT):
                ps = psum.tile([P, B], F32, tag="gates", name=f"ig{mi}")
                for kt in range(KT):
                    g = mi * KT + kt
                    nc.tensor.matmul(ps[:],
                                     toh_sb[:, g * P:(g + 1) * P],
                                     zT_sb[:, kt * B:(kt + 1) * B],
                                     start=(kt == 0), stop=(kt == KT - 1))
                l, kh = divmod(mi, KT)
                hf, kk = seg_of(kh)
                for ch in range(NCH):
                    nc.scalar.activation(
                        h_init[l][ch][hf][:, kk * BC:(kk + 1) * BC],
                        ps[:, ch * BC:(ch + 1) * BC],
                        Tanh, bias=tohb_sb[:, mi:mi + 1])

            def half_out(hbf, ch):
                """Bounce h chunk to DRAM, AllGather, gather into k-slots."""
                ag_in = dram.tile([P, BC], BF16, tag=f"agin{ch}")
                # scalar (ACT) HWDGE ring: keeps this bounce out of the sync
                # ring where the gather DMAs sit waiting on their AllGather
                # (HWDGE rings are FIFO per issuing engine - a waiting gather
                # would head-of-line-block the next chunk's bounce + trigger)
                nc.scalar.dma_start(ag_in[:], hbf[:])
                if use_ag:
                    ag_out = dram.tile([H, BC], BF16, tag=f"agout{ch}",
                                       addr_space="Shared")
                    nc.gpsimd.collective_compute(
                        "AllGather", mybir.AluOpType.bypass,
                        ins=[ag_in.opt()], outs=[ag_out.opt()],
                        replica_groups=[list(range(NC))])
                else:
                    ag_out = dram.tile([H, BC], BF16, tag=f"agout{ch}")
                    nc.sync.dma_start(ag_out[0:P, :], ag_in[:])
                # three segment-gathers: the 256KB low half lands first and
                # unblocks the next x-part's phase-1 matmuls; the high half
                # arrives as two 128KB pieces matching the k-ordered
                # consumption rate so the matmul stream never stalls mid-way
                hT_new = []
                for si, (k0, kl) in enumerate(SEGS):
                    ht = hTp.tile([P, kl * BC], BF16, tag=f"hT{ch}{si}",
                                  name=f"hT{ch}{si}")
                    nc.sync.dma_start(
                        ht[:].rearrange("p (t n) -> p t n", t=kl),
                        ag_out[k0 * P:(k0 + kl) * P, :]
                        .rearrange("(t p) n -> p t n", p=P))
                    hT_new.append(ht)
                return hT_new

            def chunk_tail(l, ps_ch, ch, want_f32, need_out=True):
                """Gate activations + state update + AG for one batch chunk."""
                gate = []
                for q, fn in enumerate((Sig, Sig, Tanh, Sig)):
                    gt = actp.tile([P, BC], F32, tag=f"gate{q}{ch}",
                                   name=f"gate{q}{ch}")
                    nc.scalar.activation(gt[:], ps_ch[q][:], fn,
                                         bias=b_sb[:, 4 * l + q:4 * l + q + 1])
                    gate.append(gt)
                i_t, f_t, g_t, o_t = gate
                t1 = actp.tile([P, BC], F32, tag=f"t1{ch}")
                t2 = actp.tile([P, BC], F32, tag=f"t2{ch}")
                nc.vector.tensor_mul(t1[:], f_t[:], c_sb[l][ch][:])
                nc.vector.tensor_mul(t2[:], i_t[:], g_t[:])
                nc.vector.tensor_add(c_sb[l][ch][:], t1[:], t2[:])
                tc_t = actp.tile([P, BC], F32, tag=f"tc{ch}")
                nc.scalar.activation(tc_t[:], c_sb[l][ch][:], Tanh)
                hf32 = None
                if want_f32:
                    hf32 = actp.tile([P, BC], F32, tag=f"hf32{ch}")
                    nc.vector.tensor_mul(hf32[:], o_t[:], tc_t[:])
                hT_new = None
                if need_out:
                    hbf = actp.tile([P, BC], BF16, tag=f"hbf{ch}")
                    nc.vector.tensor_mul(hbf[:], o_t[:], tc_t[:])
                    hT_new = half_out(hbf, ch)
                return hT_new, hf32

            def layer_step(l, x_ap, h_ap, want_f32, need_out=True):
                """x_ap/h_ap: (ch, kt) -> [P, BC] AP of the rhs k-tile."""
                ih, hh = 2 * l, 2 * l + 1
                KH = KT // 2
                ps = [[psum.tile([P, BC], F32, tag="gates", name=f"g{q}{ch}")
                       for q in range(4)] for ch in range(NCH)]
                # fully per-chunk: [h-part A, x-part A, tail A, h-part B, ...]
                # so chunk A's AllGather launches while chunk B's matmuls
                # still stream - the act/trigger tail never exposes the PE,
                # and the cc queue stays fed
                hT_new, hf32 = [None] * NCH, [None] * NCH
                for ch in range(NCH):
                    # h-part: rhs gathered one layer-step ago, always ready
                    for kt in range(KT):
                        g = hh * KT + kt
                        for q in range(4):
                            nc.tensor.matmul(
                                ps[ch][q][:],
                                w_sb[:, g * 512 + q * P: g * 512 + (q + 1) * P],
                                h_ap(ch, kt),
                                start=(kt == 0), stop=False)
                    # x-part: phase 1 kt-outer over kt 0-5, consuming the
                    # segment-gathers in arrival order; phase 2 q-outer over
                    # kt 6-7 so each gate's bank closes early (staggered)
                    # and its activation overlaps the remaining stream
                    for kt in range(KT - 2):
                        g = ih * KT + kt
                        for q in range(4):
                            nc.tensor.matmul(
                                ps[ch][q][:],
                                w_sb[:, g * 512 + q * P: g * 512 + (q + 1) * P],
                                x_ap(ch, kt),
                                start=False, stop=False)
                    for q in range(4):
                        for kt in range(KT - 2, KT):
                            g = ih * KT + kt
                            nc.tensor.matmul(
                                ps[ch][q][:],
                                w_sb[:, g * 512 + q * P: g * 512 + (q + 1) * P],
                                x_ap(ch, kt),
                                start=False, stop=(kt == KT - 1))
                    hT_new[ch], hf32[ch] = chunk_tail(l, ps[ch], ch, want_f32,
                                                      need_out)
                return hT_new, hf32

            def tile_ap(tiles):
                def ap(ch, kt):
                    si, kk = seg_of(kt)
                    return tiles[ch][si][:, kk * BC:(kk + 1) * BC]
                return ap

            x0_ap = lambda ch, kt: x0T_sb[:, kt * B + ch * BC: kt * B + (ch + 1) * BC]

            for rep in range(reps):
                x_ap = x0_ap
                h_prev = [h_init[0], h_init[1]]
                for t in range(T):
                    h0, _ = layer_step(0, x_ap, tile_ap(h_prev[0]),
                                       want_f32=False)
                    h1, hf32 = layer_step(1, tile_ap(h0), tile_ap(h_prev[1]),
                                          want_f32=True,
                                          need_out=(t < T - 1))
                    for ch in range(NCH):
                        nc.scalar.dma_start(
                            ys_ext[t * P:(t + 1) * P, ch * BC:(ch + 1) * BC],
                            hf32[ch][:])
                    x_ap = tile_ap(h1)
                    h_prev = [h0, h1]

    nc.compile()
    return nc


build = _build
prep_inputs = _prep_inputs


def gather_output(results):
    slabs = [r["ys"].reshape(T, P, B).transpose(0, 2, 1) for r in results]
    return np.ascontiguousarray(np.concatenate(slabs, axis=2)).astype(np.float32)


_CACHE = {}


def kernel(**inputs) -> np.ndarray:
    if "nc" not in _CACHE:
        _CACHE["nc"] = _build()
    nc = _CACHE["nc"]
    in_maps = _prep_inputs(**inputs)

    from concourse.bass_utils import run_bass_kernel_spmd
    res = run_bass_kernel_spmd(nc, in_maps, list(range(NC)))
    return gather_output(res.results)
